# revision 40
# baseline (speedup 1.0000x reference)
"""Distributed GQA attention kernel for Trainium2 (8 NeuronCores).

Sharding: 2-way data parallel over batch x 4-way tensor parallel over heads.
Core c handles batch b = c // 4 and head group g = c % 4 (8 q-heads, 2 kv-heads).
Each core computes a full-size partial of the output (its head group pushed
through Wo); the host sums the 4 partials per batch. No on-device collective.

Device-side layout is feature-major (Q^T/K^T: [feature partitions, T free]) so
projections consume the host-pre-transposed x^T directly, attention scores are
computed transposed (S^T[tk, tq]) so softmax(P)@V needs no transposes, and the
softmax denominator comes free from an appended ones-column on V.

Schedule: the ACT(exp)-paced attention stream leaves ~2 matmuls of PE slack
per full score block; the previous q-tile's Wo matmuls are popped one-by-one
into those gaps instead of running as dense blocks afterwards. The softmax
denominators are broadcast across partitions with a tiny ones-stationary
PE matmul (emitted one block late so the strict-FIFO PE queue never waits on
the DVE sumexp copy), replacing a DRAM round trip. Inputs are host-packed so
every load is a single contiguous-line DMA; the output is written as bf16
512KB row-blocks.
"""

import numpy as np
import ml_dtypes
from collections import deque
from contextlib import ExitStack

import concourse.bass as bass
from concourse import bacc
import concourse.mybir as mybir
import concourse.tile as tile
from concourse.bass_utils import run_bass_kernel_spmd

BF16 = mybir.dt.bfloat16
F32 = mybir.dt.float32
AF = mybir.ActivationFunctionType

P = 128
B, T, D = 2, 2048, 2048
NUM_HEADS, NUM_KV_HEADS, HD = 32, 8, 64
FQ = 512          # q features per core (8 heads x 64)
DKV = 128         # kv features per core (2 kv heads x 64)
KO = D // P       # 16 contraction tiles over d_model
NT = T // 512     # 4 tiles of 512 along T
SCALE = 1.0 / np.sqrt(HD)
ROPE_BASE = 10000.0
# local head order inside the 512 q-features: pairs (j, j+4) so that the two
# heads in partition tile j sit at bases 0/64 matching kv heads 0/1 in K^T
PERM_Q = [0, 4, 1, 5, 2, 6, 3, 7]

_nc_cache = {}


def build_nc():
    if "nc" in _nc_cache:
        return _nc_cache["nc"]
    nc = bacc.Bacc()
    xS = nc.declare_dram_parameter("xS", [P, NT, KO, 512], BF16, isOutput=False)
    wqS = nc.declare_dram_parameter("wqS", [P, KO, FQ], BF16, isOutput=False)
    wkS = nc.declare_dram_parameter("wkS", [P, KO, DKV], BF16, isOutput=False)
    wvS = nc.declare_dram_parameter("wvS", [P, KO, DKV], BF16, isOutput=False)
    woS = nc.declare_dram_parameter("woS", [P, 4, D], BF16, isOutput=False)
    cosd = nc.declare_dram_parameter("cosT", [P, T], BF16, isOutput=False)
    sind = nc.declare_dram_parameter("sinT", [P, T], BF16, isOutput=False)
    mskd = nc.declare_dram_parameter("tri", [P, P], BF16, isOutput=False)
    y = nc.declare_dram_parameter("y", [T, D], BF16, isOutput=True)

    with tile.TileContext(nc) as tc:
        with ExitStack() as ctx:
            const = ctx.enter_context(tc.tile_pool(name="const", bufs=1))
            work = ctx.enter_context(tc.tile_pool(name="work", bufs=3))
            otp = ctx.enter_context(tc.tile_pool(name="otp", bufs=2))
            pexp = ctx.enter_context(tc.tile_pool(name="pexp", bufs=8))
            rrp = ctx.enter_context(tc.tile_pool(name="rrp", bufs=2))
            yrow = ctx.enter_context(tc.tile_pool(name="yrow", bufs=2))
            big_ps = ctx.enter_context(tc.tile_pool(name="bigps", bufs=2, space="PSUM"))
            pv_ps = ctx.enter_context(tc.tile_pool(name="pvps", bufs=1, space="PSUM"))
            s_ps = ctx.enter_context(tc.tile_pool(name="sps", bufs=2, space="PSUM"))

            # ---- constant loads ----
            # small early-needed constants first, then x (K/V proj gate on
            # it), then wq; wo is deferred off the startup DMA critical path.
            # Host-packed layouts make each load one contiguous-line DMA.
            cos_sb = const.tile([P, T], BF16, tag="cos")
            sin_sb = const.tile([P, T], BF16, tag="sin")
            tri_sb = const.tile([P, P], BF16, tag="tri")
            wk_sb = const.tile([P, KO, DKV], BF16, tag="wk")
            wv_sb = const.tile([P, KO, DKV], BF16, tag="wv")
            x_sb = const.tile([P, NT, KO, 512], BF16, tag="x")
            wq_sb = const.tile([P, KO, FQ], BF16, tag="wq")
            nc.sync.dma_start(wk_sb[:], wkS[:, :, :])
            nc.sync.dma_start(x_sb[:, 0, :, :], xS[:, 0, :, :])
            nc.sync.dma_start(cos_sb[:], cosd[:])
            nc.sync.dma_start(sin_sb[:], sind[:])
            nc.sync.dma_start(wv_sb[:], wvS[:, :, :])
            nc.sync.dma_start(tri_sb[:], mskd[:])
            nc.sync.dma_start(x_sb[:, 1, :, :], xS[:, 1, :, :])
            nc.sync.dma_start(wq_sb[:], wqS[:, :, :])
            nc.sync.dma_start(x_sb[:, 2, :, :], xS[:, 2, :, :])
            nc.sync.dma_start(x_sb[:, 3, :, :], xS[:, 3, :, :])
            wo_sb = const.tile([P, 4, D], BF16, tag="wo")
            ones_sb = const.tile([1, 64], BF16, tag="ones")
            nc.gpsimd.memset(ones_sb[:], 1.0)

            def rope(dst_ap, ps, nt, tag):
                """cast psum->bf16, rotate halves, combine with cos/sin tables"""
                raw = work.tile([P, 512], BF16, tag="ropraw")
                nc.scalar.copy(raw[:], ps[:])
                rot = work.tile([P, 512], BF16, tag="roprot")
                for h in range(2):
                    b0 = h * 64
                    nc.sync.dma_start(rot[b0:b0 + 32, :], raw[b0 + 32:b0 + 64, :])
                    nc.sync.dma_start(rot[b0 + 32:b0 + 64, :], raw[b0:b0 + 32, :])
                ts = slice(nt * 512, (nt + 1) * 512)
                t1 = work.tile([P, 512], BF16, tag="ropt1")
                nc.vector.tensor_mul(t1[:], raw[:], cos_sb[:, ts])
                nc.vector.tensor_mul(rot[:], rot[:], sin_sb[:, ts])
                nc.vector.tensor_add(dst_ap, t1[:], rot[:])

            # ---- K projection + rope (feature-major K^T [128, T]) ----
            kt = const.tile([P, T], BF16, tag="kt")

            def k_proj(nt):
                ps = big_ps.tile([P, 512], F32, tag="big")
                for ko in range(KO):
                    nc.tensor.matmul(ps[:], wk_sb[:, ko, :],
                                     x_sb[:, nt, ko, :],
                                     start=(ko == 0), stop=(ko == KO - 1))
                rope(kt[:, nt * 512:(nt + 1) * 512], ps, nt, "k")

            # ---- V projection (token-major, with ones column appended) ----
            # v_sb[:, tt, 0:65] = [V_kv0 | 1], v_sb[:, tt, 65:130] = [V_kv1 | 1]
            v_sb = const.tile([P, 16, 130], BF16, tag="v")
            nc.gpsimd.memset(v_sb[:, :, 64:65], 1.0)
            nc.gpsimd.memset(v_sb[:, :, 129:130], 1.0)

            def v_proj(tt):
                ps = big_ps.tile([P, DKV], F32, tag="big")
                for ko in range(KO):
                    nc.tensor.matmul(
                        ps[:], x_sb[:, tt // 4, ko,
                                    (tt % 4) * P:(tt % 4 + 1) * P],
                        wv_sb[:, ko, :],
                        start=(ko == 0), stop=(ko == KO - 1))
                nc.vector.tensor_copy(v_sb[:, tt, 0:64], ps[:, 0:64])
                nc.vector.tensor_copy(v_sb[:, tt, 65:129], ps[:, 64:128])

            # ---- Q projection + rope for one head pair (all 4 windows, so
            # each window's rope resolves under the next window's matmuls) ----
            qts = {}

            def q_proj(j):
                qt_j = const.tile([P, T], BF16, tag=f"qt{j}", name=f"qt{j}")
                for nt in range(NT):
                    ps = big_ps.tile([P, 512], F32, tag="big")
                    for ko in range(KO):
                        nc.tensor.matmul(ps[:], wq_sb[:, ko, j * P:(j + 1) * P],
                                         x_sb[:, nt, ko, :],
                                         start=(ko == 0), stop=(ko == KO - 1))
                    rope(qt_j[:, nt * 512:(nt + 1) * 512], ps, nt, f"q{j}")
                qts[j] = qt_j

            # ---- Wo micro-op queue: one 512-col matmul (or a copy / output
            # DMA) per op, popped into the PE gaps of the ACT-paced attention
            # stream ----
            wo_q = deque()
            reserve = [0]

            def pop_wo(n):
                for _ in range(n):
                    if len(wo_q) > reserve[0]:
                        wo_q.popleft()()

            def drain_wo():
                reserve[0] = 0
                while wo_q:
                    wo_q.popleft()()

            def make_wo_ops(qt, ot):
                # per 128-row slice: 4 psum oc-groups staged into one [P, D]
                # bf16 row, written out as a single 512KB DMA
                ops = []
                for tt in range(4):
                    box = {}

                    def op_row(box=box):
                        box["ysb"] = yrow.tile([P, D], BF16, tag="ysb",
                                               name="ysb")
                    ops.append(op_row)
                    for oc in range(4):
                        def op_start(box=box, tt=tt, oc=oc, ot=ot):
                            box["ps"] = big_ps.tile([P, 512], F32, tag="big",
                                                    name="wops")
                            nc.tensor.matmul(
                                box["ps"][:], ot[:, 0, tt * P:(tt + 1) * P],
                                wo_sb[:, 0, oc * 512:(oc + 1) * 512],
                                start=True, stop=False)
                        ops.append(op_start)
                        for kf in range(1, 4):
                            def op_mid(box=box, kf=kf, tt=tt, oc=oc, ot=ot):
                                nc.tensor.matmul(
                                    box["ps"][:], ot[:, kf, tt * P:(tt + 1) * P],
                                    wo_sb[:, kf, oc * 512:(oc + 1) * 512],
                                    start=False, stop=(kf == 3))
                            ops.append(op_mid)

                        def op_cp(box=box, oc=oc):
                            nc.vector.tensor_copy(
                                box["ysb"][:, oc * 512:(oc + 1) * 512],
                                box["ps"][:])
                        ops.append(op_cp)

                    def op_out(box=box, qt=qt, tt=tt):
                        r0 = qt * 512 + tt * P
                        nc.sync.dma_start(y[r0:r0 + P, :], box["ysb"][:])
                    ops.append(op_out)
                return ops

            # deferred normalization: the PE part of a block's denominator
            # broadcast is emitted one block later (dep long resolved by
            # then), so the strict-FIFO PE queue never waits on the DVE chain
            pending_norm = [None]

            def flush_norm():
                if pending_norm[0] is not None:
                    pending_norm[0]()
                    pending_norm[0] = None

            # ---- attention for one (qt, j) head-pair into ot tile ----
            def attn_block(qt, j, ot):
                pv = pv_ps.tile([65, 1024], F32, tag="pv")
                pvv = pv[:].rearrange("p (two t) -> p two t", two=2)
                nkb = 4 * qt + 4

                def flush_pv(prev):
                    # PV matmuls for the previous kb (software pipeline: issued
                    # after the next kb's scores so PE never waits on ACT's exp
                    # of the current block). Diagonal blocks only touch output
                    # columns >= their first causally-valid query.
                    pkb, c0, pp = prev
                    ppv = pp[:].rearrange("p (two t) -> p two t", two=2)
                    nc.tensor.matmul(pv[:, c0:512], v_sb[:, pkb, 0:65],
                                     ppv[:, 0, c0:512],
                                     start=(pkb == 0), stop=(pkb == nkb - 1))
                    nc.tensor.matmul(pv[:, 512 + c0:1024], v_sb[:, pkb, 65:130],
                                     ppv[:, 1, c0:512],
                                     start=(pkb == 0), stop=(pkb == nkb - 1))

                pending = []
                for kb in range(nkb):
                    tk = slice(kb * P, (kb + 1) * P)
                    jr = kb - 4 * qt           # >= 0 on diagonal blocks
                    c0 = max(0, jr) * P        # first causally-valid column
                    tqs = slice(qt * 512 + c0, (qt + 1) * 512)
                    # one 2-bank psum tile holds both heads' scores; the two
                    # matmuls land on disjoint PE row halves and run
                    # concurrently, then a SINGLE exp (3-dim AP) and a single
                    # broadcast mask cover both halves
                    sp = s_ps.tile([P, 1024], F32, tag="s")
                    spv = sp[:].rearrange("p (two t) -> p two t", two=2)
                    nc.tensor.matmul(sp[:, c0:512], kt[0:64, tk],
                                     qts[j][0:64, tqs], start=True, stop=True)
                    nc.tensor.matmul(sp[:, 512 + c0:1024], kt[64:128, tk],
                                     qts[j][64:128, tqs], start=True, stop=True)
                    if kb == 3:
                        # by kb 3 the previous block's DVE sumexp copy has
                        # certainly retired, so these MMs never stall PE
                        flush_norm()
                    if len(pending) >= 3:
                        # 3-deep pipeline: the first PV of a block otherwise
                        # waits on the previous block's pv-psum release (DVE
                        # numerator/sumexp copies)
                        flush_pv(pending.pop(0))
                    pp = pexp.tile([P, 1024], BF16, tag="p")
                    ppv = pp[:].rearrange("p (two t) -> p two t", two=2)
                    nc.scalar.activation(ppv[:, :, c0:512], spv[:, :, c0:512],
                                         AF.Exp, scale=SCALE)
                    if jr >= 0:
                        # triangle mask on the one partially-valid block
                        nc.vector.tensor_mul(
                            ppv[:, :, c0:c0 + P], ppv[:, :, c0:c0 + P],
                            tri_sb[:, None, :].to_broadcast((P, 2, P)))
                    pending.append((kb, c0, pp))
                    if c0 == 0:
                        # full-width exp block: ~460ns of PE slack -> two Wo ops
                        pop_wo(2)
                for pr in pending:
                    flush_pv(pr)
                # fast pv release: stage both heads' numerators into ot and
                # the sumexp row out of PSUM; the denominator broadcast +
                # normalize runs deferred
                nc.vector.tensor_copy(ot[0:64, j, :], pvv[0:64, 0, :])
                nc.vector.tensor_copy(ot[64:128, j, :], pvv[0:64, 1, :])
                srb = rrp.tile([1, 1024], BF16, tag="srb")
                nc.vector.tensor_copy(srb[:], pv[64:65, :])

                def finish(j=j, ot=ot, srb=srb):
                    # PE-matmul broadcast of the denominators (no DRAM round
                    # trip, so the Sync ring and DMA lanes stay clear)
                    bc_ps = big_ps.tile([P, 512], F32, tag="big")
                    nc.tensor.matmul(bc_ps[0:64, :], ones_sb[0:1, :],
                                     srb[0:1, 0:512], start=True, stop=True)
                    nc.tensor.matmul(bc_ps[64:128, :], ones_sb[0:1, :],
                                     srb[0:1, 512:1024], start=True, stop=True)
                    rec = rrp.tile([P, 512], F32, tag="bc", name="recf")
                    nc.vector.reciprocal_approx_fast(rec[:], bc_ps[:])
                    for idx in range(2):
                        nc.vector.tensor_mul(
                            ot[idx * 64:(idx + 1) * 64, j, :],
                            ot[idx * 64:(idx + 1) * 64, j, :],
                            rec[idx * 64:(idx + 1) * 64, :])
                pending_norm[0] = finish

            # ---- emission order: K/V projections interleaved per window (V
            # fills the PE while the next x window streams in), then
            # ascending q-tiles; qt0's attention rides inside the Q
            # projections, and each tile's Wo rides inside the next (longer)
            # tile's attention stream ----
            for nt in range(NT):
                k_proj(nt)
                for tt in range(4 * nt, 4 * nt + 4):
                    v_proj(tt)

            ot_tiles = {0: otp.tile([P, 4, 512], BF16, tag="ot", name="otA")}
            for j in range(4):
                q_proj(j)
                if j == 1:
                    nc.sync.dma_start(wo_sb[:], woS[:, :, :])
                attn_block(0, j, ot_tiles[0])
            flush_norm()
            wo_q.extend(make_wo_ops(0, ot_tiles[0]))
            for qt in range(1, 4):
                reserve[0] = 42 if qt == 3 else 0
                ot_tiles[qt] = otp.tile([P, 4, 512], BF16, tag="ot",
                                        name=f"otB{qt}")
                for j in range(4):
                    attn_block(qt, j, ot_tiles[qt])
                drain_wo()
                flush_norm()
                wo_q.extend(make_wo_ops(qt, ot_tiles[qt]))
            drain_wo()

    nc.finalize()
    _nc_cache["nc"] = nc
    return nc


def make_in_maps(x, Wq, Wk, Wv, Wo):
    bf = ml_dtypes.bfloat16
    x = np.asarray(x, np.float32)
    Wq = np.asarray(Wq, np.float32)
    Wk = np.asarray(Wk, np.float32)
    Wv = np.asarray(Wv, np.float32)
    Wo = np.asarray(Wo, np.float32)

    # rope tables, [128, T]: row p covers head-dim d = p % 64
    half = HD // 2
    inv_freq = 1.0 / (ROPE_BASE ** (np.arange(half, dtype=np.float64) / half))
    pos = np.arange(T, dtype=np.float64)
    d_idx = np.arange(P) % HD
    freqs = pos[None, :] * inv_freq[d_idx % half][:, None]      # [128, T]
    cos_t = np.cos(freqs).astype(bf)
    sign = np.where(d_idx < half, -1.0, 1.0)[:, None]
    sin_t = (np.sin(freqs) * sign).astype(bf)

    # causal 0/1 triangle for the partially-valid diagonal sub-block
    pp = np.arange(P)[:, None]
    ff = np.arange(P)[None, :]
    tri = (ff >= pp).astype(bf)

    in_maps = []
    for c in range(8):
        b, g = c // 4, c % 4
        heads = [8 * g + h for h in PERM_Q]
        qrows = np.concatenate([np.arange(h * HD, (h + 1) * HD) for h in heads])
        kvrows = np.arange(2 * g * HD, (2 * g + 2) * HD)
        xT = x[b].T                                              # [D, T]
        xS = xT.reshape(KO, P, NT, 512).transpose(1, 2, 0, 3)    # [P,NT,KO,512]
        wqS = Wq[qrows, :].T.reshape(KO, P, FQ).transpose(1, 0, 2)
        wkS = Wk[kvrows, :].T.reshape(KO, P, DKV).transpose(1, 0, 2)
        wvS = Wv[kvrows, :].T.reshape(KO, P, DKV).transpose(1, 0, 2)
        woS = Wo[:, qrows].T.reshape(4, P, D).transpose(1, 0, 2)  # [P,4,D]
        in_maps.append({
            "xS": np.ascontiguousarray(xS).astype(bf),
            "wqS": np.ascontiguousarray(wqS).astype(bf),
            "wkS": np.ascontiguousarray(wkS).astype(bf),
            "wvS": np.ascontiguousarray(wvS).astype(bf),
            "woS": np.ascontiguousarray(woS).astype(bf),
            "cosT": cos_t,
            "sinT": sin_t,
            "tri": tri,
        })
    return in_maps


def combine_outputs(results):
    out = np.zeros((B, T, D), np.float32)
    for c in range(8):
        out[c // 4] += results[c]["y"].astype(np.float32)
    return out


def _ensure_ntff_hook():
    """Register the axon NTFF profile hook (antenv.axon_hooks is missing
    from this image; recreate it and wire the ctypes hook from trn_boot)."""
    import sys, types
    if "antenv.axon_hooks" in sys.modules:
        return
    m = types.ModuleType("antenv.axon_hooks")
    hook = [None]
    m.set_axon_ntff_profile_hook = lambda h: hook.__setitem__(0, h)
    m.get_axon_ntff_profile_hook = lambda: hook[0]
    sys.modules["antenv.axon_hooks"] = m
    import antenv
    antenv.axon_hooks = m
    sys.path.insert(0, "/root/.axon_site")
    from trn_agent_boot.trn_boot import _ntff_profile_via_ctypes
    m.set_axon_ntff_profile_hook(
        _ntff_profile_via_ctypes("/opt/axon/libaxon_pjrt.so"))


def kernel(x, Wq, Wk, Wv, Wo, _trace=False):
    if _trace:
        _ensure_ntff_hook()
    nc = build_nc()
    in_maps = make_in_maps(x, Wq, Wk, Wv, Wo)
    res = run_bass_kernel_spmd(nc, in_maps, core_ids=list(range(8)), trace=_trace)
    out = combine_outputs(res.results)
    if _trace:
        return out, res
    return out
